# revision 15
# baseline (speedup 1.0000x reference)
"""Trainium2 Bass kernel for nn_MeanSquareWithManifoldItem (v2, fp8).

For U,V (N,D), M,W (N,N), alpha (1,):
    recon = U @ V.T
    part1 = sum((recon - M)^2)
    part2 = alpha * (row_w@u_sq + col_w@v_sq - 2*sum(W*recon))
    out   = (part1 + part2) / N^2

Reformulation (no N x N recon materialization):
    C   = M + alpha*W
    total*N^2 = ||U V^T||_F^2 + sum(M^2) - 2*sum(V o (C^T U))
                + sum_ij W'_ij u_sq_i + sum_j colw'_j v_sq_j
    with W' = alpha*W, ||U V^T||_F^2 = sum(U^T U o V^T V)  (D x D Grams)

Sharding: rows of U, M, W split across 8 cores; V replicated.
Per-core device work (core c, rows R = [c*N/8, (c+1)*N/8)):
  - S_c = C_c^T U_c via fp8 DoubleRow matmuls (contraction over local rows),
    drained as sigma_c = sum(V o S_c) on DVE          [the only N^2*D work]
  - C built in SBUF: DMA W' then DMA M with accum_op=add (CCE, free)
  - W' stats  [colw'_j ; sum_i usq_i W'_ij] via fp8 ones-matmuls
  - sum(M^2) via ACT Square accumulate over fp8 M tiles
  - partial Grams U_c^T U_c, V_c^T V_c in bf16 (cheap, D x D out)
Host combines partial Grams/stats/scalars in float64.
"""

import sys

if "/opt/trn_rl_repo" not in sys.path:
    sys.path.insert(0, "/opt/trn_rl_repo")

import contextlib

import numpy as np
import ml_dtypes

NCORES = 8
PT = 128
JB = 1024  # column block streamed per pipeline stage


def _build(N, D, use_fp8=True, repeat=1, do_compile=True):
    from concourse import bass, bacc, mybir, tile

    f32 = mybir.dt.float32
    bf16 = mybir.dt.bfloat16
    f8 = mybir.dt.float8e4
    AF = mybir.ActivationFunctionType
    OP = mybir.AluOpType
    DR = mybir.MatmulPerfMode.DoubleRow

    ROWS = N // NCORES       # 1024 rows per core
    Q = ROWS // PT           # 8 row chunks of 128
    NRSC = ROWS // (2 * PT)  # 4 superchunks of 256 (fp8 pairs)
    NJB = N // JB            # 8 column blocks
    NC512 = JB // 512        # 2 stats chunks per block
    JC = JB // PT            # 8 main j-chunks per block
    NJC = N // PT            # 64 j-chunks total
    NDC = D // PT            # 4 gram output chunks
    NRC = ROWS // PT         # 8 gram row chunks

    nc = bacc.Bacc(
        "TRN2",
        target_bir_lowering=False,
        debug=False,
        num_devices=NCORES,
    )
    m_d = nc.declare_dram_parameter("m_rows", [ROWS, N], f8, isOutput=False)
    wp_d = nc.declare_dram_parameter("wp_rows", [ROWS, N], f8, isOutput=False)
    u8_d = nc.declare_dram_parameter("u8", [ROWS, D], f8, isOutput=False)
    vt8_d = nc.declare_dram_parameter("vt8", [D, N], f8, isOutput=False)
    ubf_d = nc.declare_dram_parameter("ubf", [ROWS, D], bf16, isOutput=False)
    vbf_d = nc.declare_dram_parameter("vbf", [ROWS, D], bf16, isOutput=False)
    ou_d = nc.declare_dram_parameter("onesusq", [PT, NRSC * 2 * 16], f8, isOutput=False)
    m2_d = nc.declare_dram_parameter("acc_m2", [PT, 1], f32, isOutput=True)
    sg_d = nc.declare_dram_parameter("acc_sig", [PT, 1], f32, isOutput=True)
    ws_d = nc.declare_dram_parameter("wstat", [2, N], f32, isOutput=True)
    gu_d = nc.declare_dram_parameter("gram_u", [PT, NDC * D], f32, isOutput=True)
    gv_d = nc.declare_dram_parameter("gram_v", [PT, NDC * D], f32, isOutput=True)

    with tile.TileContext(nc) as tc:
        with (
            tc.tile_pool(name="const", bufs=1) as constp,
            tc.tile_pool(name="tw", bufs=4) as twp,
            tc.tile_pool(name="tm", bufs=3) as tmp_,
            tc.tile_pool(name="scr", bufs=3) as scrp,
            tc.tile_pool(name="ps", bufs=4, space=bass.MemorySpace.PSUM) as psp,
            tc.tile_pool(name="pws", bufs=2, space=bass.MemorySpace.PSUM) as pwsp,
            tc.tile_pool(name="pg", bufs=2, space=bass.MemorySpace.PSUM) as pgp,
        ):
            vt8 = constp.tile([PT, NDC * N], f8)    # V^T, [p, (dc j)]
            u8 = constp.tile([PT, Q * D], f8)       # U rows, [p, (q d)]
            ubf = constp.tile([PT, NRC * D], bf16)  # U rows bf16 for Gram
            vbf = constp.tile([PT, NRC * D], bf16)  # V local rows bf16
            onesusq = constp.tile([PT, NRSC * 2 * 16], f8)
            mcols = constp.tile([PT, NJB], f32)     # ACT-written sum(M^2) cols
            sig_cols = constp.tile([PT, NJC], f32)  # DVE-written sigma cols
            wstat_sb = constp.tile([PT, N], f32)    # rows 0,1 used
            redm = constp.tile([PT, 1], f32)
            redsig = constp.tile([PT, 1], f32)

            nc.sync.dma_start(
                vt8[:].rearrange("p (dc j) -> p dc j", j=N),
                vt8_d.rearrange("(dc p) j -> p dc j", p=PT),
            )
            nc.sync.dma_start(
                u8[:].rearrange("p (q d) -> p q d", d=D),
                u8_d.rearrange("(q p) d -> p q d", p=PT),
            )
            nc.sync.dma_start(
                ubf[:].rearrange("p (rc d) -> p rc d", d=D),
                ubf_d.rearrange("(rc p) d -> p rc d", p=PT),
            )
            nc.sync.dma_start(
                vbf[:].rearrange("p (rc d) -> p rc d", d=D),
                vbf_d.rearrange("(rc p) d -> p rc d", p=PT),
            )
            nc.sync.dma_start(onesusq[:], ou_d[:])
            nc.vector.memset(mcols[:], 0.0)
            nc.vector.memset(sig_cols[:], 0.0)

            def emit_front(jb):
                """W' load + stats matmuls + wstat drain + CCE accum (C=W'+M)."""
                tw = twp.tile([PT, Q, JB], f8, tag="tw")
                nc.sync.dma_start(
                    tw[:],
                    wp_d[:, jb * JB : (jb + 1) * JB].rearrange(
                        "(q p) c -> p q c", p=PT
                    ),
                )
                for c2 in range(NC512):
                    pws = pwsp.tile([PT, 512], f32, tag="pws")
                    for r in range(NRSC):
                        nc.tensor.matmul(
                            pws[0:16, :],
                            onesusq[:, r * 32 : (r + 1) * 32].rearrange(
                                "p (two f) -> p two f", two=2
                            ),
                            tw[:, 2 * r : 2 * r + 2, c2 * 512 : (c2 + 1) * 512],
                            start=(r == 0),
                            stop=(r == NRSC - 1),
                            perf_mode=DR,
                        )
                    nc.vector.tensor_scalar_mul(
                        wstat_sb[0:2, (jb * NC512 + c2) * 512 : (jb * NC512 + c2 + 1) * 512],
                        pws[0:2, :],
                        1.0,
                    )
                nc.gpsimd.dma_start(
                    tw[:],
                    m_d[:, jb * JB : (jb + 1) * JB].rearrange(
                        "(q p) c -> p q c", p=PT
                    ),
                    accum_op=OP.add,
                )
                return tw

            def emit_gram_chunk(k):
                """One Gram output chunk (8 bf16 matmuls + drain + store)."""
                src, dst_d = ((ubf, gu_d), (vbf, gv_d))[k // NDC]
                dc = k % NDC
                pg = pgp.tile([PT, D], f32, tag="pg")
                for rc in range(NRC):
                    nc.tensor.matmul(
                        pg[:],
                        src[:, rc * D + dc * PT : rc * D + (dc + 1) * PT],
                        src[:, rc * D : (rc + 1) * D],
                        start=(rc == 0),
                        stop=(rc == NRC - 1),
                    )
                gsb = scrp.tile([PT, D], f32, tag="gsb")
                nc.scalar.activation(gsb[:], pg[:], AF.Copy)
                nc.sync.dma_start(dst_d[:, dc * D : (dc + 1) * D], gsb[:])

            rep_ctx = tc.For_i(0, repeat, 1) if repeat > 1 else None
            with rep_ctx if rep_ctx is not None else contextlib.nullcontext():
                tws = {0: emit_front(0)}
                for jb in range(NJB):
                    # pipeline: next block's front work goes ahead of this
                    # block's mains so PE never waits on the accum DMA
                    if jb + 1 < NJB:
                        tws[jb + 1] = emit_front(jb + 1)
                    # independent M copy for sum(M^2) on ACT
                    tm = tmp_.tile([PT, Q, JB], f8, tag="tm")
                    nc.sync.dma_start(
                        tm[:],
                        m_d[:, jb * JB : (jb + 1) * JB].rearrange(
                            "(q p) c -> p q c", p=PT
                        ),
                    )
                    scr2 = scrp.tile([PT, Q * JB], f8, tag="scr2")
                    nc.scalar.activation(
                        scr2[:],
                        tm[:].rearrange("p q c -> p (q c)"),
                        AF.Square,
                        accum_out=mcols[:, jb : jb + 1],
                    )
                    # one Gram chunk per block as extra PE cover for the accum
                    emit_gram_chunk(jb)
                    # --- main S^T = U^T C matmuls (U stationary, each weight
                    # load feeds both 512-col halves) + sigma drain ---
                    tw = tws.pop(jb)
                    for dc in range(NDC):
                        psA = psp.tile([PT, 512], f32, tag="ps")
                        psB = psp.tile([PT, 512], f32, tag="ps")
                        for r in range(NRSC):
                            lhsT = u8[:, r * 2 * D : (r + 1) * 2 * D].rearrange(
                                "p (two d) -> p two d", two=2
                            )[:, :, dc * PT : (dc + 1) * PT]
                            for h, px in ((0, psA), (1, psB)):
                                nc.tensor.matmul(
                                    px[:],
                                    lhsT,
                                    tw[:, 2 * r : 2 * r + 2, h * 512 : (h + 1) * 512],
                                    start=(r == 0),
                                    stop=(r == NRSC - 1),
                                    perf_mode=DR,
                                    skip_group_check=True,
                                )
                        for h, px in ((0, psA), (1, psB)):
                            slot = jb * 2 * NDC + dc * 2 + h
                            scr = scrp.tile([PT, 512], bf16, tag="scr")
                            nc.vector.scalar_tensor_tensor(
                                out=scr[:],
                                in0=px[:],
                                scalar=1.0,
                                in1=vt8[
                                    :,
                                    dc * N + jb * JB + h * 512 : dc * N + jb * JB + (h + 1) * 512,
                                ],
                                op0=OP.mult,
                                op1=OP.mult,
                                accum_out=sig_cols[:, slot : slot + 1],
                            )
                # --- final reductions + outputs ---
                nc.vector.tensor_reduce(
                    redm[:], mcols[:], mybir.AxisListType.X, OP.add
                )
                nc.vector.tensor_reduce(
                    redsig[:], sig_cols[:], mybir.AxisListType.X, OP.add
                )
                nc.sync.dma_start(m2_d[:], redm[:])
                nc.sync.dma_start(sg_d[:], redsig[:])
                nc.sync.dma_start(ws_d[:], wstat_sb[0:2, :])
    _dedupe_ldweights(nc)
    if do_compile:
        nc.compile()
    return nc


def _dedupe_ldweights(nc):
    """Drop InstLdweights that reload the exact weights already resident.

    The tile legalizer emits one Ldweights per Matmult; back-to-back
    matmuls sharing a stationary operand reload it redundantly (~213ns
    each for fp8 DoubleRow pairs). Safe to drop when the duplicate has
    no semaphore updates and its waits are identical to the kept load
    (the kept load precedes it on the same engine queue).
    """
    removed = 0
    for fn in nc.m.functions:
        for b in fn.blocks:
            insts = list(b.instructions)
            out = []
            last = None  # kept Ldweights fingerprint (ap+sync)
            for inst in insts:
                tn = type(inst).__name__
                if tn == "InstLdweights":
                    c = inst.concise()
                    key = (str(inst.ins[0]), str(inst.tile_position),
                           str(inst.tile_size), str(inst.perf_mode))
                    waits = sorted(p for p in c.split() if p.startswith("wait:"))
                    has_upd = "update:" in c
                    if (last is not None and key == last[0] and not has_upd
                            and (not waits or waits == last[1])):
                        removed += 1
                        continue
                    last = (key, waits)
                elif tn == "InstMatmult":
                    if inst.is_transpose:
                        last = None
                else:
                    pass  # non-PE instructions leave PE weights intact
                out.append(inst)
            b.instructions = out
    return removed


_CACHE = {}


def _get_nc(N, D, use_fp8=True, repeat=1):
    key = (N, D, use_fp8, repeat)
    if key not in _CACHE:
        _CACHE[key] = _build(N, D, use_fp8, repeat)
    return _CACHE[key]


def _make_in_maps(U, V, M, W, alpha):
    f8 = ml_dtypes.float8_e4m3
    bf = ml_dtypes.bfloat16
    N, D = U.shape
    ROWS = N // NCORES
    NRSC = ROWS // 256
    a = float(np.asarray(alpha).reshape(-1)[0])
    U = np.asarray(U, np.float32)
    V = np.asarray(V, np.float32)
    M8 = np.ascontiguousarray(M).astype(f8)
    Wp8 = (np.asarray(W, np.float32) * np.float32(a)).astype(f8)
    U8 = U.astype(f8)
    VT8 = (V.T).astype(f8)
    Ubf = U.astype(bf)
    Vbf = V.astype(bf)
    usq_full = (U.astype(np.float64) ** 2).sum(axis=1)
    usq8 = (usq_full / 64.0).astype(np.float32).astype(f8)

    in_maps = []
    for c in range(NCORES):
        r0, r1 = c * ROWS, (c + 1) * ROWS
        ou = np.zeros((PT, NRSC * 2 * 16), f8)
        for rsc in range(NRSC):
            for i in range(2):
                base = rsc * 32 + i * 16
                ou[:, base + 0] = f8(1.0)
                ou[:, base + 1] = usq8[r0 + rsc * 256 + i * 128 : r0 + rsc * 256 + (i + 1) * 128]
        in_maps.append(
            {
                "m_rows": M8[r0:r1],
                "wp_rows": Wp8[r0:r1],
                "u8": np.ascontiguousarray(U8[r0:r1]),
                "vt8": VT8,
                "ubf": np.ascontiguousarray(Ubf[r0:r1]),
                "vbf": np.ascontiguousarray(Vbf[r0:r1]),
                "onesusq": ou,
            }
        )
    return in_maps


def _combine(res, V, N, D):
    NDC = D // PT
    vsq_full = (np.asarray(V, np.float32).astype(np.float64) ** 2).sum(axis=1)
    m2 = 0.0
    sig = 0.0
    gu = np.zeros((D, D), np.float64)
    gv = np.zeros((D, D), np.float64)
    ws = np.zeros((2, N), np.float64)
    for r in res:
        m2 += r["acc_m2"].astype(np.float64).sum()
        sig += r["acc_sig"].astype(np.float64).sum()
        gu += r["gram_u"].astype(np.float64).reshape(PT, NDC, D).transpose(1, 0, 2).reshape(D, D)
        gv += r["gram_v"].astype(np.float64).reshape(PT, NDC, D).transpose(1, 0, 2).reshape(D, D)
        ws += r["wstat"].astype(np.float64)
    rec2 = float((gu * gv).sum())
    colwp = ws[0]
    rwu = float(ws[1].sum()) * 64.0
    cwv = float(colwp @ vsq_full)
    total = (rec2 + m2 - 2.0 * sig + rwu + cwv) / (float(N) * float(N))
    return np.float32(total)


def run(U, V, M, W, alpha, trace=False):
    from concourse.bass_utils import run_bass_kernel_spmd

    N, D = np.asarray(U).shape
    nc = _get_nc(N, D, True)
    in_maps = _make_in_maps(U, V, M, W, alpha)
    bkr = run_bass_kernel_spmd(nc, in_maps, list(range(NCORES)), trace=trace)
    return _combine(bkr.results, V, N, D), bkr


def kernel(U, V, M, W, alpha):
    out, _ = run(U, V, M, W, alpha)
    return np.asarray(out, dtype=np.float32)


def bench(U, V, M, W, alpha, use_bf16=True, iters=20, warmup=3, repeat=1):
    """Steady-state per-execution timing with device-resident inputs."""
    import jax
    from jax.sharding import Mesh, PartitionSpec, NamedSharding
    from jax.experimental.shard_map import shard_map
    from concourse import mybir
    from concourse import bass2jax as b2j

    N, D = np.asarray(U).shape
    nc = _get_nc(N, D, True, repeat)
    b2j.install_neuronx_cc_hook()

    in_maps = _make_in_maps(U, V, M, W, alpha)
    partition_name = nc.partition_id_tensor.name if nc.partition_id_tensor else None

    in_names, out_names, out_avals, zero_outs = [], [], [], []
    for alloc in nc.m.functions[0].allocations:
        if not isinstance(alloc, mybir.MemoryLocationSet):
            continue
        name = alloc.memorylocations[0].name
        if alloc.kind == "ExternalInput":
            if name != partition_name:
                in_names.append(name)
        elif alloc.kind == "ExternalOutput":
            out_names.append(name)
            shape = tuple(alloc.tensor_shape)
            dtype = mybir.dt.np(alloc.dtype)
            out_avals.append(jax.core.ShapedArray(shape, dtype))
            zero_outs.append(np.zeros(shape, dtype))
    n_params = len(in_names)
    all_in_names = list(in_names) + out_names
    if partition_name is not None:
        all_in_names.append(partition_name)

    def _body(*args):
        operands = list(args)
        if partition_name is not None:
            operands.append(b2j.partition_id_tensor())
        outs = b2j._bass_exec_p.bind(
            *operands,
            out_avals=tuple(out_avals),
            in_names=tuple(all_in_names),
            out_names=tuple(out_names),
            lowering_input_output_aliases=(),
            sim_require_finite=True,
            sim_require_nnan=True,
            nc=nc,
        )
        return tuple(outs)

    devices = jax.devices()[:NCORES]
    mesh = Mesh(np.asarray(devices), ("core",))
    nshard = NamedSharding(mesh, PartitionSpec("core"))
    in_specs = (PartitionSpec("core"),) * (n_params + len(out_names))
    out_specs = (PartitionSpec("core"),) * len(out_names)
    sharded = jax.jit(
        shard_map(_body, mesh=mesh, in_specs=in_specs, out_specs=out_specs,
                  check_rep=False),
        keep_unused=True,
    )

    concat_in = [
        np.concatenate([np.asarray(in_maps[c][nm]) for c in range(NCORES)], axis=0)
        for nm in in_names
    ]
    concat_zeros = [
        np.zeros((NCORES * z.shape[0], *z.shape[1:]), z.dtype) for z in zero_outs
    ]
    dev_args = [jax.device_put(a, nshard) for a in concat_in + concat_zeros]

    import time

    for _ in range(warmup):
        outs = sharded(*dev_args)
    jax.block_until_ready(outs)
    t0 = time.perf_counter()
    for _ in range(iters):
        outs = sharded(*dev_args)
    jax.block_until_ready(outs)
    dt = (time.perf_counter() - t0) / iters

    res = [
        {
            nm: np.asarray(outs[i]).reshape(NCORES, *out_avals[i].shape)[c]
            for i, nm in enumerate(out_names)
        }
        for c in range(NCORES)
    ]
    return dt, _combine(res, V, N, D)
